# revision 12
# baseline (speedup 1.0000x reference)
"""Trainium2 Bass kernel for nn_Block_75067438400016 (moe_routing).

Transformer block: pre-LN causal MHA (B=4,T=2048,D=128,H=8,HS=16)
+ pre-LN top-2-of-8 MoE FFN (FF=512) + switch aux loss.

Sharding (8 cores, SPMD single NEFF):
  - Attention: (batch b = c//2, head-group g = c%2 -> 4 heads/core), full
    causal T per core. Scores computed transposed (S^T[k,q]) so exp output
    p^T feeds PV directly; row-sums come free via ones-columns appended to V.
  - Output projection produces partial sums over the head dim; a pairwise
    ReduceScatter (groups [2c,2c+1]) both reduces and reshards to 1024
    tokens/core.
  - MoE: dense per-expert FFN on the core's 1024 tokens; top-2 gates are
    broadcast along partitions via a DRAM round-trip; combine fuses
    (+b2, *gate) with scalar_tensor_tensor.
All heavy matmuls run in bf16 with fp32 PSUM accumulation; LN stats,
softmax row-sums, router and residuals stay fp32.
"""

import os
import sys

import numpy as np

for _p in ("/opt/trn_rl_repo",):
    if _p not in sys.path and os.path.isdir(_p):
        sys.path.insert(0, _p)

import ml_dtypes

import concourse.bass as bass
from concourse import bacc as _bacc
import concourse.mybir as mybir
import concourse.tile as tile
from concourse.bass import ds, ts
from concourse.bass_utils import run_bass_kernel_spmd
from concourse.masks import make_identity

F32 = mybir.dt.float32
BF16 = mybir.dt.bfloat16
AF = mybir.ActivationFunctionType
ALU = mybir.AluOpType

B, T, D, H = 4, 2048, 128, 8
HS = D // H          # 16
E, TOPK = 8, 2
FF = 4 * D           # 512
EPS = 1e-5
P = 128
TBF = T // P         # 16 token blocks (full seq)
THALF = T // 2       # 1024 tokens per core after reduce-scatter
TBH = THALF // P     # 8 token blocks (half seq)
GH = 4               # heads per core
QC = 512             # query chunk
NQC = T // QC        # 4
FFC = FF // P        # 4 ff chunks

LAST_RESULT = None   # BassKernelResults of the most recent run (for test.py)



def _bcast_ap(handle, n_part):
    a = handle[:] if not isinstance(handle, bass.AP) else handle
    return bass.AP(tensor=a.tensor, offset=a.offset, ap=[[0, n_part]] + list(a.ap))

def build_bass(debug=False):
    nc = _bacc.Bacc(None, num_devices=8)

    # ---- I/O ----
    x_b = nc.dram_tensor("x_b", [T, D], F32, kind="ExternalInput")
    x_half = nc.dram_tensor("x_half", [THALF, D], F32, kind="ExternalInput")
    wqk = nc.dram_tensor("wqk", [2, D, P], BF16, kind="ExternalInput")   # spread q|k
    qkbias = nc.dram_tensor("qkbias", [2, P], F32, kind="ExternalInput")
    wv = nc.dram_tensor("wv", [D, 64], BF16, kind="ExternalInput")       # packed v
    vbias = nc.dram_tensor("vbias", [64], F32, kind="ExternalInput")
    wproj = nc.dram_tensor("wproj", [P, D], BF16, kind="ExternalInput")
    bproj = nc.dram_tensor("bproj", [D], F32, kind="ExternalInput")
    wgate = nc.dram_tensor("wgate", [D, E], F32, kind="ExternalInput")
    bgate = nc.dram_tensor("bgate", [E], F32, kind="ExternalInput")
    w1 = nc.dram_tensor("w1", [D, E, FF], BF16, kind="ExternalInput")    # [d,e,(cc p)]
    b1 = nc.dram_tensor("b1", [P, E, FFC], F32, kind="ExternalInput")
    w2 = nc.dram_tensor("w2", [P, E, FFC, D], BF16, kind="ExternalInput")
    b2 = nc.dram_tensor("b2", [P, E], F32, kind="ExternalInput")

    y_out = nc.dram_tensor("y_out", [THALF, D], F32, kind="ExternalOutput")
    aux_out = nc.dram_tensor("aux_out", [1, 16], F32, kind="ExternalOutput")

    dbg = {}
    if debug:
        dbg["xn"] = nc.dram_tensor("dbg_xn", [T, D], BF16, kind="ExternalOutput")
        dbg["qT"] = nc.dram_tensor("dbg_qT", [P, T], BF16, kind="ExternalOutput")
        dbg["kT"] = nc.dram_tensor("dbg_kT", [P, T], BF16, kind="ExternalOutput")
        dbg["attnT"] = nc.dram_tensor("dbg_attnT", [P, T], BF16, kind="ExternalOutput")
        dbg["ypart"] = nc.dram_tensor("dbg_ypart", [T, D], BF16, kind="ExternalOutput")
        dbg["xmid"] = nc.dram_tensor("dbg_xmid", [THALF, D], F32, kind="ExternalOutput")
        dbg["probs"] = nc.dram_tensor("dbg_probs", [THALF, E], F32, kind="ExternalOutput")
        dbg["gwf"] = nc.dram_tensor("dbg_gwf", [THALF, E], F32, kind="ExternalOutput")
        dbg["ffT"] = nc.dram_tensor("dbg_ffT", [P, THALF], F32, kind="ExternalOutput")

    cc_in = nc.dram_tensor("cc_in", [T, D], BF16)                 # internal
    cc_out = nc.dram_tensor("cc_out", [THALF, D], BF16)
    gw_dram = nc.dram_tensor("gw_dram", [E, THALF], F32)          # internal

    with tile.TileContext(nc) as tc:
        with (
            tc.tile_pool(name="singles", bufs=1) as singles,
            tc.tile_pool(name="work", bufs=3) as work,
        ):
            # ---------- persistent SBUF ----------
            ident = singles.tile([P, P], F32)
            make_identity(nc, ident)
            ident_bf = singles.tile([P, P], BF16)
            make_identity(nc, ident_bf)
            eps_sb = singles.tile([P, 1], F32)
            nc.vector.memset(eps_sb, EPS)
            ones_sb = singles.tile([P, 1], F32)
            nc.vector.memset(ones_sb, 1.0)

            x_sb = singles.tile([P, TBF, D], F32)
            nc.sync.dma_start(out=x_sb, in_=x_b[:].rearrange("(tb p) d -> p tb d", p=P))
            xh_sb = singles.tile([P, TBH, D], F32)
            nc.sync.dma_start(out=xh_sb, in_=x_half[:].rearrange("(tb p) d -> p tb d", p=P))

            wqk_sb = singles.tile([D, 2, P], BF16)
            nc.sync.dma_start(out=wqk_sb, in_=wqk[:].rearrange("a d p -> d a p"))
            qkb_sb = singles.tile([P, 2], F32)
            nc.sync.dma_start(out=qkb_sb, in_=qkbias[:].rearrange("a p -> p a"))
            wv_sb = singles.tile([D, 64], BF16)
            nc.sync.dma_start(out=wv_sb, in_=wv[:])
            vb_bc = singles.tile([P, 64], F32)
            nc.sync.dma_start(
                out=vb_bc,
in_=_bcast_ap(vbias, P),
            )
            wproj_sb = singles.tile([P, D], BF16)
            nc.sync.dma_start(out=wproj_sb, in_=wproj[:])
            bproj_bc = singles.tile([P, D], F32)
            nc.sync.dma_start(
                out=bproj_bc,
in_=_bcast_ap(bproj, P),
            )
            wgate_sb = singles.tile([D, E], F32)
            nc.sync.dma_start(out=wgate_sb, in_=wgate[:])
            bgate_bc = singles.tile([P, E], F32)
            nc.sync.dma_start(
                out=bgate_bc,
in_=_bcast_ap(bgate, P),
            )
            w1_sb = singles.tile([D, E, FFC, P], BF16)
            nc.sync.dma_start(out=w1_sb, in_=w1[:].rearrange("d e (c p) -> d e c p", p=P))
            b1_sb = singles.tile([P, E, FFC], F32)
            nc.sync.dma_start(out=b1_sb, in_=b1[:])
            w2_sb = singles.tile([P, E, FFC, D], BF16)
            nc.sync.dma_start(out=w2_sb, in_=w2[:])
            b2_sb = singles.tile([P, E], F32)
            nc.sync.dma_start(out=b2_sb, in_=b2[:])

            # warm-up touches: first consumption of DMA-loaded tiles happens
            # here on each engine so later compute instrs carry <=1 wait each
            wsc = singles.tile([P, 4], F32)
            for _ap in (qkb_sb[:, 0:1], vb_bc[:, 0:1], bproj_bc[:, 0:1],
                        bgate_bc[:, 0:1], b2_sb[:, 0:1], xh_sb[:, 0, 0:1],
                        x_sb[:, 0, 0:1], wgate_sb[:, 0:1]):
                nc.vector.tensor_copy(out=wsc[:, 0:1], in_=_ap)
            wsc2 = singles.tile([P, 4], F32)
            nc.scalar.copy(out=wsc2[:, 0:1], in_=b1_sb[:, 0, 0:1])

            xn_bf = singles.tile([P, TBF, D], BF16)
            xnT = singles.tile([D, T], BF16)
            qT_sb = singles.tile([P, T], BF16)
            kT_sb = singles.tile([P, T], BF16)
            v_sb = singles.tile([P, TBF, GH, HS], BF16)
            ones16 = singles.tile([P, HS], BF16)
            nc.vector.memset(ones16, 1.0)
            attnT_sb = singles.tile([P, T], BF16)
            ypart_bf = singles.tile([P, TBF, D], BF16)
            xmid = singles.tile([P, TBH, D], F32)
            xn2_bf = singles.tile([P, TBH, D], BF16)
            xn2f = singles.tile([P, TBH, D], F32)
            xn2T = singles.tile([D, THALF], BF16)
            xn2Tf = singles.tile([D, THALF], F32)
            probs = singles.tile([P, TBH, E], F32)
            gwf = singles.tile([P, TBH, E], F32)
            ffaccT = singles.tile([D, THALF], F32)
            y_sb = singles.tile([P, TBH, D], F32)
            aux_sb = singles.tile([1, 16], F32)

            # ================= Phase A: LN1 + QKV + attention + proj =================
            with (
                tc.tile_pool(name="pse", bufs=2, space="PSUM") as pse,
                tc.tile_pool(name="pst", bufs=2, space="PSUM") as pst,
            ):
                # ---- LN1 (no gamma/beta: folded into weights) ----
                mv1 = singles.tile([P, TBF, 2], F32)
                for tb in range(TBF):
                    st1 = work.tile([P, 6], F32)
                    nc.vector.bn_stats(out=st1, in_=x_sb[:, tb, :])
                    nc.vector.bn_aggr(out=mv1[:, tb, :], in_=st1)
                lnv1 = singles.tile([P, TBF], F32)
                nc.scalar.activation(out=lnv1, in_=mv1[:, :, 1], func=AF.Ln,
                                     bias=eps_sb, scale=1.0)
                rstd1 = singles.tile([P, TBF], F32)
                nc.scalar.activation(out=rstd1, in_=lnv1, func=AF.Exp,
                                     bias=0.0, scale=-0.5)
                for tb in range(TBF):
                    nc.vector.tensor_scalar(
                        out=xn_bf[:, tb, :], in0=x_sb[:, tb, :],
                        scalar1=mv1[:, tb, 0:1], scalar2=rstd1[:, tb:tb + 1],
                        op0=ALU.subtract, op1=ALU.mult)
                for tb in range(TBF):
                    ptx = pst.tile([P, P], BF16, tag="ptx")
                    nc.tensor.transpose(out=ptx, in_=xn_bf[:, tb, :],
                                        identity=ident_bf)
                    nc.vector.tensor_copy(out=xnT[:, ts(tb, P)], in_=ptx)
                if debug:
                    nc.gpsimd.dma_start(
                        out=dbg["xn"][:].rearrange("(tb p) d -> p tb d", p=P), in_=xn_bf)

                # ---- q^T / k^T (spread: head h at partitions 32h..32h+16) ----
                for i2, dst in ((0, qT_sb), (1, kT_sb)):
                    for ch in range(T // 512):
                        pq = pse.tile([P, 512], F32, tag="pse")
                        nc.tensor.matmul(pq, lhsT=wqk_sb[:, i2, :],
                                         rhs=xnT[:, ts(ch, 512)])
                        nc.vector.tensor_scalar_add(
                            out=dst[:, ts(ch, 512)], in0=pq,
                            scalar1=qkb_sb[:, i2:i2 + 1])
                # ---- v (packed cols 16h+j) + ones cols ----
                nc.vector.memset(attnT_sb, 0.0)
                for tb in range(TBF):
                    pv = pse.tile([P, 512], F32, tag="pse")
                    nc.tensor.matmul(pv[:, :64], lhsT=xnT[:, ts(tb, P)], rhs=wv_sb)
                    nc.vector.tensor_tensor(
                        out=v_sb[:, tb, :, 0:16],
                        in0=pv[:, :64].rearrange("p (h j) -> p h j", h=GH),
                        in1=vb_bc.rearrange("p (h j) -> p h j", h=GH),
                        op=ALU.add)
                if debug:
                    nc.gpsimd.dma_start(out=dbg["qT"][:], in_=qT_sb)
                    nc.gpsimd.dma_start(out=dbg["kT"][:], in_=kT_sb)

            # ================= Phase A1: attention =================
            with (
                tc.tile_pool(name="psS", bufs=1, space="PSUM") as psS,
                tc.tile_pool(name="psAt", bufs=1, space="PSUM") as psAt,
                tc.tile_pool(name="psP", bufs=2, space="PSUM") as psP,
                tc.tile_pool(name="att_w", bufs=3) as att_w,
            ):
                # ---- attention (scores^T -> exp -> mask -> PV) ----
                for qc in range(NQC):
                    qs = qc * QC
                    nb = qs // P + 4
                    at_ps = psAt.tile([P, QC], F32, tag="at")
                    at_sum = psAt.tile([P, QC], F32, tag="atsum")
                    for kb in range(nb):
                        j = kb - (nb - 4)     # >=0 : diagonal block index
                        c0 = max(0, j) * P
                        w = QC - c0
                        sc = psS.tile([P, GH, 512], F32, tag="sc")
                        for h in range(GH):
                            nc.tensor.matmul(
                                sc[:, h, c0:QC],
                                lhsT=kT_sb[32 * h:32 * h + HS, ts(kb, P)],
                                rhs=qT_sb[32 * h:32 * h + HS, ds(qs + c0, w)],
                                tile_position=(32 * h, 0))
                        pT = att_w.tile([P, GH, 512], BF16, tag="pT")
                        nc.scalar.activation(out=pT[:, :, c0:QC], in_=sc[:, :, c0:QC],
                                             func=AF.Exp)
                        if j >= 0:
                            nc.gpsimd.affine_select(
                                out=pT[:, :, c0:c0 + P], in_=pT[:, :, c0:c0 + P],
                                pattern=[[0, GH], [1, P]], compare_op=ALU.is_ge,
                                fill=0.0, base=0, channel_multiplier=-1)
                        for h in range(GH):
                            nc.tensor.matmul(
                                at_ps[32 * h:32 * h + HS, c0:QC],
                                lhsT=v_sb[:, kb, h, :],
                                rhs=pT[:, h, c0:QC],
                                start=(kb == 0), stop=(kb == nb - 1),
                                tile_position=(0, 32 * h))
                        for h in range(GH):
                            nc.tensor.matmul(
                                at_sum[32 * h:32 * h + HS, c0:QC],
                                lhsT=ones16,
                                rhs=pT[:, h, c0:QC],
                                start=(kb == 0), stop=(kb == nb - 1),
                                tile_position=(0, 32 * h))
                    rec = att_w.tile([P, QC], F32, tag="rec")
                    nc.vector.reciprocal_approx_fast(out=rec, in_=at_sum)
                    for h in range(GH):
                        nc.vector.tensor_tensor(
                            out=attnT_sb[32 * h:32 * h + HS, ds(qs, QC)],
                            in0=at_ps[32 * h:32 * h + HS, :],
                            in1=rec[32 * h:32 * h + HS, :],
                            op=ALU.mult)
                if debug:
                    nc.gpsimd.dma_start(out=dbg["attnT"][:], in_=attnT_sb)

                # ---- projection partials ----
                for tb in range(TBF):
                    py = psP.tile([P, P], F32, tag="py")
                    nc.tensor.matmul(py, lhsT=attnT_sb[:, ts(tb, P)],
                                     rhs=wproj_sb)
                    nc.vector.tensor_copy(out=ypart_bf[:, tb, :], in_=py)
                nc.gpsimd.dma_start(out=cc_in[:].rearrange("(tb p) d -> p tb d", p=P),
                                  in_=ypart_bf)
                if debug:
                    nc.gpsimd.dma_start(
                        out=dbg["ypart"][:].rearrange("(tb p) d -> p tb d", p=P),
                        in_=ypart_bf)

            # ---- pairwise reduce-scatter on proj partials ----
            nc.gpsimd.collective_compute(
                "ReduceScatter", ALU.add,
                replica_groups=[[0, 1], [2, 3], [4, 5], [6, 7]],
                ins=[cc_in[:]], outs=[cc_out[:]])

            # ================= Phase A2: residual + LN2 + transposes =================
            with (
                tc.tile_pool(name="pst2", bufs=2, space="PSUM") as pst2,
            ):
                cco_sb = singles.tile([P, TBH, D], BF16)
                nc.gpsimd.dma_start(out=cco_sb,
                                  in_=cc_out[:].rearrange("(tb p) d -> p tb d", p=P))
                nc.vector.tensor_copy(out=wsc[:, 1:2], in_=cco_sb[:, 0, 0:1])
                for tb in range(TBH):
                    nc.vector.tensor_tensor(out=xmid[:, tb, :], in0=xh_sb[:, tb, :],
                                            in1=cco_sb[:, tb, :], op=ALU.add)
                    nc.vector.tensor_tensor(out=xmid[:, tb, :], in0=xmid[:, tb, :],
                                            in1=bproj_bc, op=ALU.add)
                if debug:
                    nc.gpsimd.dma_start(
                        out=dbg["xmid"][:].rearrange("(tb p) d -> p tb d", p=P), in_=xmid)

                # ---- LN2 ----
                mv2 = singles.tile([P, TBH, 2], F32)
                for tb in range(TBH):
                    st2 = work.tile([P, 6], F32)
                    nc.vector.bn_stats(out=st2, in_=xmid[:, tb, :])
                    nc.vector.bn_aggr(out=mv2[:, tb, :], in_=st2)
                lnv2 = singles.tile([P, TBH], F32)
                nc.scalar.activation(out=lnv2, in_=mv2[:, :, 1], func=AF.Ln,
                                     bias=eps_sb, scale=1.0)
                rstd2 = singles.tile([P, TBH], F32)
                nc.scalar.activation(out=rstd2, in_=lnv2, func=AF.Exp,
                                     bias=0.0, scale=-0.5)
                for tb in range(TBH):
                    nc.vector.tensor_scalar(
                        out=xn2_bf[:, tb, :], in0=xmid[:, tb, :],
                        scalar1=mv2[:, tb, 0:1], scalar2=rstd2[:, tb:tb + 1],
                        op0=ALU.subtract, op1=ALU.mult)
                    nc.vector.tensor_scalar(
                        out=xn2f[:, tb, :], in0=xmid[:, tb, :],
                        scalar1=mv2[:, tb, 0:1], scalar2=rstd2[:, tb:tb + 1],
                        op0=ALU.subtract, op1=ALU.mult)
                for tb in range(TBH):
                    ptb = pst2.tile([P, P], BF16, tag="ptb")
                    nc.tensor.transpose(out=ptb, in_=xn2_bf[:, tb, :],
                                        identity=ident_bf)
                    nc.vector.tensor_copy(out=xn2T[:, ts(tb, P)], in_=ptb)
                for tb in range(TBH):
                    ptr = pst2.tile([P, P], F32, tag="ptr")
                    nc.tensor.transpose(out=ptr, in_=xn2f[:, tb, :], identity=ident)
                    nc.vector.tensor_copy(out=xn2Tf[:, ts(tb, P)], in_=ptr)

            # ================= Phase B: router + MoE =================
            with (
                tc.tile_pool(name="psh", bufs=2, space="PSUM") as psh,
                tc.tile_pool(name="pseo", bufs=1, space="PSUM") as pseo,
                tc.tile_pool(name="psm", bufs=2, space="PSUM") as psm,
                tc.tile_pool(name="moe_w", bufs=2) as moe_w,
            ):

                # ---- router (fp32) ----
                plg = psm.tile([P, P], F32, tag="psm")
                for tb in range(TBH):
                    nc.tensor.matmul(plg[:, ts(tb, E)], lhsT=xn2Tf[:, ts(tb, P)],
                                     rhs=wgate_sb)
                logits = singles.tile([P, TBH, E], F32)
                nc.vector.tensor_tensor(
                    out=logits, in0=plg[:, :TBH * E].rearrange(
                        "p (tb e) -> p tb e", e=E),
                    in1=bgate_bc[:, None, :].to_broadcast([P, TBH, E]), op=ALU.add)
                probs_u = singles.tile([P, TBH, E], F32)
                nc.scalar.activation(out=probs_u, in_=logits, func=AF.Exp)
                rs = work.tile([P, TBH], F32, tag="rs")
                nc.vector.reduce_sum(out=rs, in_=probs_u, axis=mybir.AxisListType.X)
                rsi = work.tile([P, TBH], F32, tag="rsi")
                nc.vector.reciprocal(out=rsi, in_=rs)
                nc.vector.tensor_tensor(out=probs, in0=probs_u,
                                        in1=rsi[:, :, None].to_broadcast([P, TBH, E]),
                                        op=ALU.mult)
                # ---- top-2 ----
                m1 = work.tile([P, TBH], F32, tag="m1")
                nc.vector.reduce_max(out=m1, in_=probs, axis=mybir.AxisListType.X)
                eqm = work.tile([P, TBH, E], F32, tag="eqm")
                nc.vector.tensor_tensor(out=eqm, in0=probs,
                                        in1=m1[:, :, None].to_broadcast([P, TBH, E]),
                                        op=ALU.is_equal)
                pm = work.tile([P, TBH, E], F32, tag="pm")
                nc.vector.scalar_tensor_tensor(out=pm, in0=eqm, scalar=-2.0,
                                               op0=ALU.mult, in1=probs, op1=ALU.add)
                m2 = work.tile([P, TBH], F32, tag="m2")
                nc.vector.reduce_max(out=m2, in_=pm, axis=mybir.AxisListType.X)
                gem = singles.tile([P, TBH, E], F32)
                nc.vector.tensor_tensor(out=gem, in0=probs,
                                        in1=m2[:, :, None].to_broadcast([P, TBH, E]),
                                        op=ALU.is_ge)
                den = work.tile([P, TBH], F32, tag="den")
                nc.vector.tensor_tensor(out=den, in0=m1, in1=m2, op=ALU.add)
                dinv = work.tile([P, TBH], F32, tag="dinv")
                nc.vector.reciprocal(out=dinv, in_=den)
                gsel = work.tile([P, TBH, E], F32, tag="gsel")
                nc.vector.tensor_tensor(out=gsel, in0=probs, in1=gem, op=ALU.mult)
                nc.vector.tensor_tensor(out=gwf, in0=gsel,
                                        in1=dinv[:, :, None].to_broadcast([P, TBH, E]),
                                        op=ALU.mult)
                if debug:
                    nc.gpsimd.dma_start(
                        out=dbg["probs"][:].rearrange("(tb p) e -> p tb e", p=P),
                        in_=probs)
                    nc.gpsimd.dma_start(
                        out=dbg["gwf"][:].rearrange("(tb p) e -> p tb e", p=P), in_=gwf)

                # ---- aux partials: f (counts via gem), p (sum probs) ----
                pfa = psm.tile([P, P], F32, tag="psm")
                nc.tensor.matmul(pfa[:1, :TBH * E], lhsT=ones_sb,
                                 rhs=gem.rearrange("p tb e -> p (tb e)"))
                nc.vector.tensor_reduce(
                    out=aux_sb[:, 0:E],
                    in_=pfa[:1, :TBH * E].rearrange("p (tb e) -> p e tb", e=E),
                    axis=mybir.AxisListType.X, op=ALU.add)
                ppa = psm.tile([P, P], F32, tag="psm")
                nc.tensor.matmul(ppa[:1, :TBH * E], lhsT=ones_sb,
                                 rhs=probs.rearrange("p tb e -> p (tb e)"))
                nc.vector.tensor_reduce(
                    out=aux_sb[:, E:2 * E],
                    in_=ppa[:1, :TBH * E].rearrange("p (tb e) -> p e tb", e=E),
                    axis=mybir.AxisListType.X, op=ALU.add)
                nc.gpsimd.dma_start(out=aux_out[:], in_=aux_sb)

                # ---- gates -> DRAM -> partition-broadcast tiles ----
                pgt = psm.tile([P, P], F32, tag="psm")
                nc.tensor.transpose(out=pgt[:TBH * E, :],
                                    in_=gwf.rearrange("p tb e -> p (tb e)"),
                                    identity=ident)
                gwT_sb = singles.tile([TBH * E, P], F32)
                nc.vector.tensor_copy(out=gwT_sb, in_=pgt[:TBH * E, :])
                for e in range(E):
                    nc.gpsimd.dma_start(out=gw_dram[e, :], in_=gwT_sb[e:TBH * E:E, :])

                # ---- dense experts ----
                for e in range(E):
                    gbc = moe_w.tile([P, THALF], F32, tag="gbc")
                    nc.gpsimd.dma_start(
                        out=gbc,
                        in_=_bcast_ap(gw_dram[e], P))
                    nc.vector.tensor_copy(out=wsc[:, 2:3], in_=gbc[:, 0:1])
                    hT = moe_w.tile([P, FFC, THALF], BF16, tag="hT")
                    for cc in range(FFC):
                        ph = psh.tile([P, THALF], F32, tag="ph")
                        for th in range(THALF // 512):
                            nc.tensor.matmul(ph[:, ts(th, 512)],
                                             lhsT=w1_sb[:, e, cc, :],
                                             rhs=xn2T[:, ts(th, 512)])
                        nc.scalar.activation(out=hT[:, cc, :], in_=ph, func=AF.Gelu,
                                             bias=b1_sb[:, e, cc:cc + 1], scale=1.0)
                    peo = pseo.tile([P, THALF], F32, tag="peo")
                    for cc in range(FFC):
                        for th in range(THALF // 512):
                            nc.tensor.matmul(peo[:, ts(th, 512)],
                                             lhsT=w2_sb[:, e, cc, :],
                                             rhs=hT[:, cc, ts(th, 512)],
                                             start=(cc == 0), stop=(cc == FFC - 1))
                    tmp = moe_w.tile([P, THALF], BF16, tag="tmp")
                    nc.vector.scalar_tensor_tensor(
                        out=tmp, in0=peo, scalar=b2_sb[:, e:e + 1], op0=ALU.add,
                        in1=gbc, op1=ALU.mult)
                    if e == 0:
                        nc.vector.tensor_copy(out=ffaccT, in_=tmp)
                    else:
                        nc.vector.tensor_tensor(out=ffaccT, in0=ffaccT, in1=tmp,
                                                op=ALU.add)
                if debug:
                    nc.gpsimd.dma_start(out=dbg["ffT"][:], in_=ffaccT)

                # ---- final: y = xmid + ff ----
                for tb in range(TBH):
                    pyf = psm.tile([P, P], F32, tag="psm")
                    nc.tensor.transpose(out=pyf, in_=ffaccT[:, ts(tb, P)],
                                        identity=ident)
                    nc.vector.tensor_tensor(out=y_sb[:, tb, :], in0=pyf,
                                            in1=xmid[:, tb, :], op=ALU.add)
                nc.gpsimd.dma_start(out=y_out[:].rearrange("(tb p) d -> p tb d", p=P),
                                  in_=y_sb)

    return nc


_BUILT = {}


def _get_nc(debug=False):
    if debug not in _BUILT:
        nc = build_bass(debug)
        nc.finalize()
        _BUILT[debug] = nc
    return _BUILT[debug]


def _prep_inputs(inputs):
    """Host-side weight folding + per-core input maps."""
    f = lambda k: np.asarray(inputs[k], np.float32)
    x = f("x")
    ln1_g, ln1_b = f("ln1_g"), f("ln1_b")
    wq, wk, wv = f("wq"), f("wk"), f("wv")
    w_proj, b_proj = f("w_proj"), f("b_proj")
    ln2_g, ln2_b = f("ln2_g"), f("ln2_b")
    w_gate = f("w_gate")
    w1, b1 = f("w1"), f("b1")
    w2, b2 = f("w2"), f("b2")

    bf = ml_dtypes.bfloat16
    scale = 1.0 / np.sqrt(np.float32(HS))

    # fold ln1 gamma into q/k/v weights; bias -> post-matmul additive vectors
    wq_g = ln1_g[None, :, None] * wq * scale      # [H,D,HS]
    wk_g = ln1_g[None, :, None] * wk
    wv_g = ln1_g[None, :, None] * wv
    bq = np.einsum("d,hde->he", ln1_b, wq * scale)  # [H,HS]
    bk = np.einsum("d,hde->he", ln1_b, wk)
    bv = np.einsum("d,hde->he", ln1_b, wv)

    # fold ln2 gamma into gate/w1
    wgate_f = ln2_g[:, None] * w_gate             # [D,E]
    bgate_f = ln2_b @ w_gate                      # [E]
    w1_f = ln2_g[None, :, None] * w1              # [E,D,FF]
    b1_f = b1 + np.einsum("d,edf->ef", ln2_b, w1)  # [E,FF]

    w1_pack = np.ascontiguousarray(w1_f.transpose(1, 0, 2)).astype(bf)  # [D,E,FF]
    b1_pack = np.ascontiguousarray(
        b1_f.reshape(E, FFC, P).transpose(2, 0, 1)).astype(np.float32)  # [P,E,FFC]
    w2_pack = np.ascontiguousarray(
        w2.reshape(E, FFC, P, D).transpose(2, 0, 1, 3)).astype(bf)      # [P,E,FFC,D]
    b2_pack = np.ascontiguousarray(b2.T).astype(np.float32)             # [P->D?,E]

    in_maps = []
    for c in range(8):
        b_idx, g = c // 2, c % 2
        heads = range(4 * g, 4 * g + 4)
        wqk_c = np.zeros((2, D, P), np.float32)
        qkb_c = np.zeros((2, P), np.float32)
        for hh, hglob in enumerate(heads):
            wqk_c[0, :, 32 * hh:32 * hh + HS] = wq_g[hglob]
            wqk_c[1, :, 32 * hh:32 * hh + HS] = wk_g[hglob]
            qkb_c[0, 32 * hh:32 * hh + HS] = bq[hglob]
            qkb_c[1, 32 * hh:32 * hh + HS] = bk[hglob]
        wv_c = np.zeros((D, 64), np.float32)
        vb_c = np.zeros((64,), np.float32)
        for hh, hglob in enumerate(heads):
            wv_c[:, 16 * hh:16 * hh + HS] = wv_g[hglob]
            vb_c[16 * hh:16 * hh + HS] = bv[hglob]
        wproj_c = np.zeros((P, D), np.float32)
        for hh, hglob in enumerate(heads):
            wproj_c[32 * hh:32 * hh + HS, :] = w_proj[16 * hglob:16 * hglob + HS, :]

        in_maps.append({
            "x_b": np.ascontiguousarray(x[b_idx]).astype(np.float32),
            "x_half": np.ascontiguousarray(
                x[b_idx, THALF * g:THALF * (g + 1)]).astype(np.float32),
            "wqk": wqk_c.astype(bf),
            "qkbias": qkb_c,
            "wv": wv_c.astype(bf),
            "vbias": vb_c,
            "wproj": np.ascontiguousarray(wproj_c).astype(bf),
            "bproj": b_proj,
            "wgate": np.ascontiguousarray(wgate_f),
            "bgate": bgate_f,
            "w1": w1_pack,
            "b1": b1_pack,
            "w2": w2_pack,
            "b2": b2_pack,
        })
    return in_maps


def kernel(**inputs):
    global LAST_RESULT
    debug = bool(int(os.environ.get("KERNEL_DEBUG", "0")))
    nc = _get_nc(debug)
    in_maps = _prep_inputs(inputs)
    res = run_bass_kernel_spmd(nc, in_maps, core_ids=list(range(8)))
    LAST_RESULT = res

    y = np.zeros((B, T, D), np.float32)
    f_tot = np.zeros(E, np.float64)
    p_tot = np.zeros(E, np.float64)
    for c in range(8):
        b_idx, g = c // 2, c % 2
        out = res.results[c]
        y[b_idx, THALF * g:THALF * (g + 1)] = out["y_out"]
        f_tot += out["aux_out"][0, :E].astype(np.float64)
        p_tot += out["aux_out"][0, E:].astype(np.float64)
    n_tok = B * T
    f_e = f_tot / (n_tok * TOPK)
    p_e = p_tot / n_tok
    aux = np.float32(E * np.sum(f_e * p_e))
    return y, aux
